# revision 21
# baseline (speedup 1.0000x reference)
"""Multi-head attention forward (B=32, N=1024, E=768, H=12) on 8 NeuronCores.

Sharding: data-parallel over batch — each core computes 4 full batches.
No collectives. All matmuls in bf16 with fp32 PSUM accumulation.

Layout strategy (per core):
  - x is DMA-transposed once into xT [e, t] (bf16).
  - q, k are produced directly in transposed layout qT/kT [feat, tok]
    (lhsT = w_qkv chunk, rhs = xT)  -> exactly what scores need.
  - v is produced in natural layout [tok, feat] (lhsT = xT chunk,
    rhs = w_qkv columns), stored per 128-token chunk as [128, 12, 65]
    where column 64 of each head is constant 1.0 (softmax-sum trick).
  - scoresT [kt, qt] = kT.T @ qT (K=64; even/odd heads occupy
    complementary halves of the PE array -> run concurrently).
  - expT = exp(0.125 * scoresT) on ScalarE straight out of PSUM (logits
    are O(1), so no max subtraction is needed).
  - attn@v: out[0:64] = v.T @ expT, out[64] = sum(expT) via the ones
    column.  Normalization happens after: out * (1/sum) with the
    reciprocal broadcast across partitions by a stride-0 DMA.
  - proj consumes the transposed attention output directly (K=128) and
    emits natural-layout [tok, 768] fp32 + bias -> DMA to DRAM.
"""

import sys

sys.path.insert(0, "/opt/trn_rl_repo")

import numpy as np

import concourse.bass as bass
import concourse.bacc as bacc
import concourse.tile as tile
import concourse.mybir as mybir

B, N, E, H, D = 32, 1024, 768, 12, 64
N_CORES = 8
B_CORE = B // N_CORES  # batches per core
F32 = mybir.dt.float32
BF16 = mybir.dt.bfloat16
AF = mybir.ActivationFunctionType

EC = E // 128            # 6 contraction chunks over the embedding dim
FC_QK = (2 * E) // 128   # 12 output chunks (128 rows each) covering q|k
TT = N // 128            # 8 token chunks of 128 per batch
QS = N // 512            # 2 query slices of 512 per batch
KC = N // 128            # 8 key chunks of 128 per batch
SCALE = 1.0 / float(np.sqrt(D))


def _pbcast(ap, parts):
    """Partition-stride-0 broadcast view: replicate a [1, n] (or 1-D DRAM
    vector) access pattern across `parts` partitions."""
    free = list(ap.ap)
    if len(free) > 1 and free[0][1] == 1:
        free = free[1:]  # drop the single-partition dim
    return bass.AP(tensor=ap.tensor, offset=ap.offset, ap=[[0, parts]] + free)


def build_kernel(b_core=B_CORE):
    nc = bacc.Bacc(None, target_bir_lowering=False, debug=False)
    tok = b_core * N

    xt_d = nc.dram_tensor("xt", [E, tok], BF16, kind="ExternalInput")
    wqkv_d = nc.dram_tensor("w_qkv", [E, 3 * E], BF16, kind="ExternalInput")
    bqkv_d = nc.dram_tensor("b_qkv", [3 * E], F32, kind="ExternalInput")
    wproj_d = nc.dram_tensor("w_proj", [E, E], BF16, kind="ExternalInput")
    bproj_d = nc.dram_tensor("b_proj", [E], F32, kind="ExternalInput")
    out_d = nc.dram_tensor("out", [tok, E], F32, kind="ExternalOutput")

    from contextlib import ExitStack

    with ExitStack() as ctx:
        tc = ctx.enter_context(tile.TileContext(nc))
        ep = ctx.enter_context
        wq_pool = ep(tc.tile_pool(name="wq", bufs=1))          # 6x [128,2304] bf16
        wp_pool = ep(tc.tile_pool(name="wp", bufs=1))          # 6x [128,768]  bf16
        bias_pool = ep(tc.tile_pool(name="bias", bufs=1))
        xt_pool = ep(tc.tile_pool(name="xt", bufs=2))          # [128,6,1024] bf16
        qk_pool = ep(tc.tile_pool(name="qk", bufs=19))         # [128,1024] bf16
        v_pool = ep(tc.tile_pool(name="v", bufs=11))           # [128,12,65] bf16
        expt_pool = ep(tc.tile_pool(name="expt", bufs=6))     # [128,1024] bf16
        aout_pool = ep(tc.tile_pool(name="aout", bufs=2))      # [128,6,1024] bf16
        ostage_pool = ep(tc.tile_pool(name="ostage", bufs=2))  # [128,768] f32
        norm_pool = ep(tc.tile_pool(name="norm", bufs=3))      # small fp32 tiles
        otmp_pool = ep(tc.tile_pool(name="otmp", bufs=2))      # [64,512] bf16
        ps_sc = ep(tc.tile_pool(name="ps_sc", bufs=1, space="PSUM"))   # [128,1024]
        ps_mm = ep(tc.tile_pool(name="ps_mm", bufs=2, space="PSUM"))   # [128,512]
        ps_av = ep(tc.tile_pool(name="ps_av", bufs=4, space="PSUM"))   # [65,512]
        if True:
            # ---------------- setup: weights + biases ----------------
            wq_sb = []   # per e-chunk: [128, 2304] bf16
            for ec in range(EC):
                wt = wq_pool.tile([128, 3 * E], BF16, tag=f"wq{ec}")
                nc.sync.dma_start(out=wt, in_=wqkv_d[ec * 128:(ec + 1) * 128, :])
                wq_sb.append(wt)

            wp_sb = []   # per 128-row chunk of w_proj: [128, 768] bf16
            for j in range(EC):
                wt = wp_pool.tile([128, E], BF16, tag=f"wp{j}")
                nc.sync.dma_start(out=wt, in_=wproj_d[j * 128:(j + 1) * 128, :])
                wp_sb.append(wt)

            # q,k biases: per-partition scalars, one column per 128-row chunk
            bqk_pp = bias_pool.tile([128, FC_QK], F32, tag="bqk")
            for fc in range(FC_QK):
                nc.gpsimd.dma_start(
                    out=bqk_pp[:, fc:fc + 1], in_=bqkv_d[fc * 128:(fc + 1) * 128]
                )
            # v / proj biases: broadcast across partitions (vary along free dim)
            bv_bc = bias_pool.tile([128, E], F32, tag="bv")
            nc.gpsimd.dma_start(out=bv_bc, in_=_pbcast(bqkv_d[2 * E:3 * E], 128))
            bp_bc = bias_pool.tile([128, E], F32, tag="bp")
            nc.gpsimd.dma_start(out=bp_bc, in_=_pbcast(bproj_d[:], 128))

            # ---------------- emission units ----------------
            # Emission order doubles as a software pipeline: the attention of
            # batch b is ScalarE(exp)-bound, so between its score chunks we
            # emit next batch's QKV and previous batch's proj matmuls as
            # "fillers" to keep the PE busy (and the HAM clock gate at 8/8).
            st = {"xt": {}, "qk": {}, "v": {}, "aout": {}, "ost": {}}

            def emit_xt(b):
                t0 = b * N
                xt_sb = xt_pool.tile([128, EC, N], BF16, tag="xt", name=f"xt{b}")
                st["xt"][b] = xt_sb
                for ec in range(EC):
                    nc.sync.dma_start(
                        out=xt_sb[:, ec, :],
                        in_=xt_d[ec * 128:(ec + 1) * 128, t0:t0 + N],
                    )

            def qk_unit(b, fc, ts):
                # q,k rows [fc*128,(fc+1)*128) x tokens [ts*512,(ts+1)*512)
                xt_sb = st["xt"][b]
                d = st["qk"].setdefault(b, {})
                if fc not in d:
                    d[fc] = qk_pool.tile([128, N], BF16, tag="qk", name=f"qk{b}_{fc}")
                qt = d[fc]
                ps = ps_mm.tile([128, 512], F32, tag="mm", name="psqk")
                for ec in range(EC):
                    nc.tensor.matmul(
                        ps,
                        wq_sb[ec][:, fc * 128:(fc + 1) * 128],
                        xt_sb[:, ec, ts * 512:(ts + 1) * 512],
                        start=(ec == 0),
                        stop=(ec == EC - 1),
                    )
                nc.vector.tensor_scalar_add(
                    qt[:, ts * 512:(ts + 1) * 512], ps, bqk_pp[:, fc:fc + 1]
                )

            def v_unit(b, tt, fsi):
                # v for tokens [tt*128,(tt+1)*128), feature half fsi
                xt_sb = st["xt"][b]
                d = st["v"].setdefault(b, {})
                if tt not in d:
                    d[tt] = v_pool.tile([128, H, D + 1], BF16, tag="v",
                                        name=f"v{b}_{tt}")
                    nc.vector.memset(d[tt][:, :, D:D + 1], 1.0)
                vt = d[tt]
                fs0, fn = (0, 512) if fsi == 0 else (512, 256)
                ps = ps_mm.tile([128, 512], F32, tag="mm", name="psv")
                for ec in range(EC):
                    nc.tensor.matmul(
                        ps[:, :fn],
                        xt_sb[:, ec, tt * 128:(tt + 1) * 128],
                        wq_sb[ec][:, 2 * E + fs0: 2 * E + fs0 + fn],
                        start=(ec == 0),
                        stop=(ec == EC - 1),
                    )
                h0, nh = fs0 // D, fn // D
                nc.vector.tensor_add(
                    vt[:, h0:h0 + nh, 0:D],
                    ps[:, :fn].rearrange("p (h d) -> p h d", d=D),
                    bv_bc[:, fs0: fs0 + fn].rearrange("p (h d) -> p h d", d=D),
                )

            def proj_unit(b, tt, fsi):
                aout = st["aout"][b]
                if (b, tt) not in st["ost"]:
                    st["ost"][(b, tt)] = ostage_pool.tile(
                        [128, E], F32, tag="ost", name=f"ost{b}_{tt}"
                    )
                ost = st["ost"][(b, tt)]
                fs0, fn = (0, 512) if fsi == 0 else (512, 256)
                ps = ps_mm.tile([128, 512], F32, tag="mm", name="psproj")
                for j in range(EC):
                    nc.tensor.matmul(
                        ps[:, :fn],
                        aout[:, j, tt * 128:(tt + 1) * 128],
                        wp_sb[j][:, fs0:fs0 + fn],
                        start=(j == 0),
                        stop=(j == EC - 1),
                    )
                nc.vector.tensor_add(
                    ost[:, fs0:fs0 + fn], ps[:, :fn], bp_bc[:, fs0:fs0 + fn]
                )
                if fsi == 1:
                    t0 = b * N
                    nc.sync.dma_start(
                        out=out_d[t0 + tt * 128: t0 + (tt + 1) * 128, :], in_=ost
                    )
                    del st["ost"][(b, tt)]

            def emit_attn(b, hp, fillers):
                if b not in st["aout"]:
                    st["aout"][b] = aout_pool.tile(
                        [128, EC, N], BF16, tag="aout", name=f"aout{b}"
                    )
                aout = st["aout"][b]
                qt_t = st["qk"][b][hp]        # q heads 2hp (parts 0:64), 2hp+1
                kt_t = st["qk"][b][EC + hp]   # k heads likewise
                v_sb = st["v"][b]
                # 4 open attn@v accumulation chains: (parity, query-slice)
                av = {}
                for par in range(2):
                    for qs in range(QS):
                        av[(par, qs)] = ps_av.tile(
                            [D + 1, 512], F32, tag="av", name=f"psav{par}{qs}"
                        )
                for kc in range(KC):
                    for par in range(2):      # even / odd head of the pair
                        p0 = par * 64
                        h = 2 * hp + par
                        ps = ps_sc.tile([128, N], F32, tag="sc", name="pssc")
                        for qs in range(QS):
                            nc.tensor.matmul(
                                ps[:, qs * 512:(qs + 1) * 512],
                                kt_t[p0:p0 + 64, kc * 128:(kc + 1) * 128],
                                qt_t[p0:p0 + 64, qs * 512:(qs + 1) * 512],
                                start=True,
                                stop=True,
                            )
                        et = expt_pool.tile([128, N], BF16, tag="expt", name="et")
                        nc.scalar.activation(et, ps, AF.Exp, scale=SCALE)
                        for qs in range(QS):
                            nc.tensor.matmul(
                                av[(par, qs)][0:D + 1, :],
                                v_sb[kc][:, h, :],
                                et[:, qs * 512:(qs + 1) * 512],
                                start=(kc == 0),
                                stop=(kc == KC - 1),
                            )
                    if fillers:
                        fillers.popleft()()

                for par in range(2):
                    for qs in range(QS):
                        ps = av[(par, qs)]
                        # Copy the [65, 512] accumulator to SBUF right away so
                        # the PSUM bank frees quickly (the normalization chain
                        # below has ~8us of latency and must not hold PSUM
                        # hostage, or the PE starves and HAM re-throttles).
                        uo = norm_pool.tile([D + 1, 512], F32, tag="uo")
                        nc.vector.tensor_copy(uo, ps[0:D + 1, :])
                        # normalize: out[0:64] * (1 / out[64]).  The exact DVE
                        # reciprocal costs ~6 cycles/elem serially, so spread
                        # the 512 sums over 64 partitions via DMA, invert
                        # 64-wide, spread back, then partition-broadcast.
                        spr = norm_pool.tile([64, 8], F32, tag="spr")
                        nc.gpsimd.dma_start(out=spr, in_=uo[D:D + 1, :])
                        nc.vector.reciprocal(spr, spr)
                        rbc = norm_pool.tile([64, 512], F32, tag="rbc")
                        nc.gpsimd.dma_start(out=rbc[0:1, :], in_=spr)
                        nc.gpsimd.partition_broadcast(rbc[:, :], rbc[0:1, :])
                        if par == 0:
                            nc.vector.tensor_mul(
                                aout[0:64, hp, qs * 512:(qs + 1) * 512],
                                uo[0:D, :],
                                rbc,
                            )
                        else:
                            ot = otmp_pool.tile([64, 512], BF16, tag="otmp")
                            nc.vector.tensor_mul(ot, uo[0:D, :], rbc)
                            nc.gpsimd.dma_start(
                                out=aout[64:128, hp, qs * 512:(qs + 1) * 512],
                                in_=ot,
                            )
                    if fillers:
                        fillers.popleft()()

            # ---------------- pipelined schedule ----------------
            from collections import deque
            from functools import partial

            def weave(b_next, b_prev):
                qk = [partial(qk_unit, b_next, fc, ts)
                      for fc in range(FC_QK) for ts in range(QS)] if b_next is not None else []
                vv = [partial(v_unit, b_next, tt, fsi)
                      for tt in range(TT) for fsi in range(2)] if b_next is not None else []
                pj = [partial(proj_unit, b_prev, tt, fsi)
                      for tt in range(TT) for fsi in range(2)] if b_prev is not None else []
                out = []
                while qk or vv or pj:
                    if qk: out.append(qk.pop(0))
                    if vv: out.append(vv.pop(0))
                    if pj: out.append(pj.pop(0))
                    if qk: out.append(qk.pop(0))
                return deque(out)

            emit_xt(0)
            for fc in range(FC_QK):
                for ts in range(QS):
                    qk_unit(0, fc, ts)
            for tt in range(TT):
                for fsi in range(2):
                    v_unit(0, tt, fsi)

            for b in range(b_core):
                b_next = b + 1 if b + 1 < b_core else None
                b_prev = b - 1 if b >= 1 else None
                if b_next is not None:
                    emit_xt(b_next)
                fillers = weave(b_next, b_prev)
                for hp in range(H // 2):
                    emit_attn(b, hp, fillers)
                while fillers:
                    fillers.popleft()()

            for tt in range(TT):
                for fsi in range(2):
                    proj_unit(b_core - 1, tt, fsi)

    nc.compile()
    return nc


_CACHED = {}


def _get_nc(b_core=B_CORE):
    if b_core not in _CACHED:
        _CACHED[b_core] = build_kernel(b_core)
    return _CACHED[b_core]


class _Runner:
    """Cached jitted SPMD executable (mirrors bass2jax.run_bass_via_pjrt's
    multi-core path, but built once and reused across calls)."""

    def __init__(self, nc, n_chain=1):
        import jax
        import jax.numpy as jnp
        from jax.experimental.shard_map import shard_map
        from jax.sharding import Mesh, PartitionSpec
        from concourse import bass2jax, mybir

        bass2jax.install_neuronx_cc_hook()
        self.jax = jax
        part_name = nc.partition_id_tensor.name if nc.partition_id_tensor else None
        in_names, out_names, out_avals, zero_shapes = [], [], [], []
        for alloc in nc.m.functions[0].allocations:
            if not isinstance(alloc, mybir.MemoryLocationSet):
                continue
            name = alloc.memorylocations[0].name
            if alloc.kind == "ExternalInput":
                if name != part_name:
                    in_names.append(name)
            elif alloc.kind == "ExternalOutput":
                out_names.append(name)
                out_avals.append(
                    jax.core.ShapedArray(alloc.tensor_shape, mybir.dt.np(alloc.dtype))
                )
                zero_shapes.append((tuple(alloc.tensor_shape), mybir.dt.np(alloc.dtype)))
        self.in_names = list(in_names)
        self.out_names = out_names
        self.out_avals = out_avals
        self.zero_shapes = zero_shapes
        n_params = len(in_names)
        all_names = in_names + out_names
        if part_name is not None:
            all_names = all_names + [part_name]

        def _exec(*args):
            if part_name is not None:
                args = args + (bass2jax.partition_id_tensor(),)
            outs = bass2jax._bass_exec_p.bind(
                *args,
                out_avals=tuple(out_avals),
                in_names=tuple(all_names),
                out_names=tuple(out_names),
                lowering_input_output_aliases=(),
                sim_require_finite=True,
                sim_require_nnan=True,
                nc=nc,
            )
            return tuple(outs)

        xi = 0

        def _body(*args):
            ins = list(args[:n_params])
            zeros = list(args[n_params:])
            outs = _exec(*ins, *zeros)
            for _ in range(n_chain - 1):
                ins2 = list(ins)
                ins2[xi] = outs[0]  # chain: feed output back as x
                outs = _exec(*ins2, *zeros)
            return outs

        devices = jax.devices()[:N_CORES]
        mesh = Mesh(np.asarray(devices), ("core",))
        n_outs = len(out_names)
        self.fn = jax.jit(
            shard_map(
                _body,
                mesh=mesh,
                in_specs=(PartitionSpec("core"),) * (n_params + n_outs),
                out_specs=(PartitionSpec("core"),) * n_outs,
                check_rep=False,
            ),
            keep_unused=True,
        )

    def run_dev(self, dev_args):
        out = self.fn(*dev_args)
        self.jax.block_until_ready(out)
        return out

    def prep(self, in_map_global):
        """in_map_global: name -> concatenated (n_cores*dim0, ...) array."""
        args = [np.asarray(in_map_global[n]) for n in self.in_names]
        for shp, dt in self.zero_shapes:
            args.append(np.zeros((N_CORES * shp[0], *shp[1:]), dt))
        return [self.jax.device_put(a) for a in args]


_RUNNERS = {}


def _get_runner(n_chain=1):
    if n_chain not in _RUNNERS:
        _RUNNERS[n_chain] = _Runner(_get_nc(), n_chain)
    return _RUNNERS[n_chain]


def _core_inputs(x, w_qkv, b_qkv, w_proj, b_proj, c):
    """Per-core input map.  x is pre-cast to bf16 and pre-transposed to
    [E, tok] on the host — a data-layout choice that lets the kernel feed
    the PE directly (weights likewise pre-cast to bf16)."""
    import ml_dtypes

    bf = ml_dtypes.bfloat16
    shard = np.asarray(x, np.float32)[c * B_CORE:(c + 1) * B_CORE]
    xt = np.ascontiguousarray(shard.reshape(B_CORE * N, E).T.astype(bf))
    return {
        "xt": xt,
        "w_qkv": np.asarray(w_qkv).astype(bf),
        "b_qkv": np.asarray(b_qkv, np.float32),
        "w_proj": np.asarray(w_proj).astype(bf),
        "b_proj": np.asarray(b_proj, np.float32),
    }


def _global_inputs(x, w_qkv, b_qkv, w_proj, b_proj):
    per_core = [_core_inputs(x, w_qkv, b_qkv, w_proj, b_proj, c)
                for c in range(N_CORES)]
    return {
        name: np.concatenate([m[name] for m in per_core], axis=0)
        for name in per_core[0]
    }


def kernel(x, w_qkv, b_qkv, w_proj, b_proj):
    runner = _get_runner()
    dev_args = runner.prep(_global_inputs(x, w_qkv, b_qkv, w_proj, b_proj))
    outs = runner.run_dev(dev_args)
    out = np.asarray(outs[0]).reshape(B, N, E)
    return out
